# revision 34
# baseline (speedup 1.0000x reference)
"""Masked (expander) linear layer on 8 Trainium2 NeuronCores.

Computes out = x @ (W * M)^T for
  x: [16384, 2048] f32, W: [2048, 2048] f32, M: [2048, 2048] int32 (0/1)

Sharding: pure data-parallel over rows of x. Each of the 8 cores gets 2048
rows of x plus a replicated copy of W and M, computes its [2048, 2048]
output shard (transposed) locally, and the host transposes + concatenates.
No collectives.

Device-side design (v5):
 - Operands in bf16 (host casts x and W; mask as int8). bf16 matmul
   streams at 1 row/cycle and its 2-byte LD_WEIGHTS fully hides under
   the previous matmul's 512-row stream, so the steady-state matmul
   period is the 216ns floor (512 rows @ 2.4GHz). PSUM accumulates f32;
   rel err ~2e-3.
 - Orientation: y^T = (W*M) @ x^T. Stationary operand = [128,128] piece
   of the masked weight, moving operand = 512-row chunk of x^T; a
   [128,512] PSUM group accumulates over the 16 k-tiles.
 - The startup phase is per-core HBM-bandwidth-bound (x 8.4MB + panel-0
   W/M 3.15MB at ~358 GB/s), so panel 0 is processed x-CHUNK-major:
   phase mc needs only x chunk mc (2.1MB) plus the panel-0 masked
   weights (resident after phase 0), keeping demand ~within the HBM
   rate from the first matmul on. x is host-pre-tiled chunk-major so
   slices arrive in exactly the consumption order; W and masks are
   host-pre-tiled to the SBUF layout [panel, partition, kt*n] (2-4KB
   contiguous lines). Panel-0 piece 0 is split per-k so k0's weights
   land ~4us in; panel t+1 streams during panel t. Panels 1-3 run
   sub-major on the fully-resident x.
 - A memset + 8 warm-up matmuls emitted FIRST keep the PE busy from
   ~1.5us so the clock p-state is ramped before real work.
 - The DVE only does mask multiplies (2.3us per [128,2048] piece at
   1 elem/cycle - keep it off the critical path). All PSUM evacuations
   run on ScalarE (ACT) so a scheduler-reordered DVE mult can never
   block a PSUM bank handoff; only the final sub-sweep's evacs
   alternate ACT/DVE to shorten the drain tail. y stores alternate the
   scalar/sync DMA rings.
"""

from contextlib import ExitStack

import ml_dtypes
import numpy as np

import concourse.bacc as bacc
import concourse.bass as bass
import concourse.mybir as mybir
import concourse.tile as tile
from concourse.bass_utils import run_bass_kernel_spmd

N_CORES = 8
P = 128

FULL_N, FULL_OUT, FULL_IN = 16384, 2048, 2048


def build_nc(
    rows: int = FULL_N // N_CORES,
    in_dim: int = FULL_IN,
    out_dim: int = FULL_OUT,
    n_panel: int = 512,
    warm_mms: int = 14,
):
    """Per-core Bass module: yt[out, rows] = (wt * m) contracted with x.

    DRAM layouts: x chunk-major pre-tiled [P, MC*KT*512] bf16; wt/mk
    pre-tiled [NTP, P, KT*n_panel] (bf16 / int8); yt [out_dim, rows] f32.
    """
    assert rows % 512 == 0 and in_dim % P == 0 and out_dim % n_panel == 0
    KT = in_dim // P  # 16 k-tiles
    NTP = out_dim // n_panel  # 4 weight panels
    SUBS = n_panel // P  # 4 stationary sub-tiles per panel
    MC = rows // 512  # 4 moving row-chunks
    KQ = 4  # k-tiles per W/mask DMA piece
    NKQ = KT // KQ
    FW = KQ * n_panel  # flat free width of a W/mask piece

    bf16 = mybir.dt.bfloat16

    nc = bacc.Bacc("TRN2", target_bir_lowering=False, debug=False)
    x = nc.dram_tensor("x", [P, MC * KT * 512], bf16, kind="ExternalInput")
    wt = nc.dram_tensor("wt", [NTP, P, KT * n_panel], bf16, kind="ExternalInput")
    mk = nc.dram_tensor("mk", [NTP, P, KT * n_panel], mybir.dt.int8, kind="ExternalInput")
    yt = nc.dram_tensor("yt", [out_dim, rows], mybir.dt.float32, kind="ExternalOutput")

    with ExitStack() as ctx:
        tc = ctx.enter_context(tile.TileContext(nc))
        xt_pool = ctx.enter_context(tc.tile_pool(name="xt", bufs=1))
        wm_pool = ctx.enter_context(tc.tile_pool(name="wm", bufs=1))
        # Staging depth 4 = one panel in flight. The WAR on a reused buffer
        # makes panel t+1's triggers fire only as panel t's mults retire —
        # a natural throttle that keeps prefetch traffic from competing
        # with the opening phase's HBM-critical x stream. (Only mask/x
        # triggers on gpsimd and W/y on sync sit behind these waits;
        # nothing PE-critical does.)
        ws_pool = ctx.enter_context(tc.tile_pool(name="ws", bufs=4))
        msk_pool = ctx.enter_context(tc.tile_pool(name="msk", bufs=4))
        yo_pool = ctx.enter_context(tc.tile_pool(name="yo", bufs=8))
        wrm_pool = ctx.enter_context(tc.tile_pool(name="wrm", bufs=1))
        pm_pool = ctx.enter_context(tc.tile_pool(name="pm", bufs=1, space="PSUM"))

        # Warm-up first in emission order: memzero has no deps, so the
        # scheduler can start it (and the warm matmuls) immediately. It
        # runs on ACT so the DVE queue is mask-multiplies-only.
        warm = wrm_pool.tile([P, 512], bf16, tag="warm", name="warm")
        nc.scalar.memzero(warm[:])
        wpm = pm_pool.tile([P, 512], mybir.dt.float32, tag="pm7", name="pmw")
        for _ in range(warm_mms):
            nc.tensor.matmul(wpm[:], warm[:, :P], warm[:], start=True, stop=True)

        # Resident x^T, chunk-major flat: slice (mc, k) at (mc*KT+k)*512.
        xt = xt_pool.tile([P, MC * KT * 512], bf16, tag="xt", name="xt")

        def xsl(mc, k):
            return slice((mc * KT + k) * 512, (mc * KT + k + 1) * 512)

        def load_x_granule(mc, k0, nk):
            # One SWDGE trigger per nk k-slices: triggers cost ~0.6us of
            # GPSIMD queue time each, and the flat chunk-major layout makes
            # the granule contiguous (nk*1KB lines per partition).
            gsl = slice((mc * KT + k0) * 512, (mc * KT + k0 + nk) * 512)
            nc.gpsimd.dma_start(out=xt[:, gsl], in_=x[:, gsl])

        # Masked-weight tiles: one per (panel, kq), [P, KQ*n_panel] bf16
        # flat, double-buffered across panels via the tag's t%2.
        wm_t = [
            [
                wm_pool.tile([P, FW], bf16, tag=f"wm{t % 2}_{q}", name=f"wm{t}_{q}")
                for q in range(NKQ)
            ]
            for t in range(NTP)
        ]

        def load_w_piece(t, q, dma_split, mult_split=None):
            """dma_split: DMA triggers per piece (each ~0.6us of queue time);
            mult_split: DVE multiplies per piece (>= dma_split shortens the
            first k-window's readiness without extra triggers)."""
            mult_split = mult_split or dma_split
            wstage = ws_pool.tile([P, FW], bf16, tag="ws")
            mtile = msk_pool.tile([P, FW], mybir.dt.int8, tag="mt")
            cw = FW // dma_split
            for c in range(dma_split):
                csl = slice(c * cw, (c + 1) * cw)
                dsl = slice(q * FW + c * cw, q * FW + (c + 1) * cw)
                nc.sync.dma_start(out=wstage[:, csl], in_=wt[t, :, dsl])
                # Masks ride the SWDGE ring: a DMA trigger that waits (ring
                # tracking, staging WAR) stalls its whole in-order engine
                # queue, and gpsimd is the only queue with nothing
                # PE-critical behind it after the x granules land. The ACT
                # queue stays copies-only so PSUM banks always drain.
                nc.gpsimd.dma_start(out=mtile[:, csl], in_=mk[t, :, dsl])
                mw = cw // (mult_split // dma_split)
                for m0 in range(c * cw, (c + 1) * cw, mw):
                    msl = slice(m0, m0 + mw)
                    nc.vector.tensor_mul(
                        wm_t[t][q][:, msl], wstage[:, msl], mtile[:, msl]
                    )

        # DMA issue order == consumption order, W-leaning: each wm piece
        # needs its DVE mult after the W DMA, so W pieces lead their
        # k-window's x granules (chunks 0 AND 1 — the opening phase runs
        # k-major over both, halving bytes-per-flop vs a single chunk).
        load_x_granule(0, 0, 2)
        load_w_piece(0, 0, 2, 4)
        load_x_granule(0, 2, 2)
        load_w_piece(0, 1, 2)
        load_x_granule(0, 4, KQ)
        load_x_granule(1, 4, KQ)
        load_w_piece(0, 2, 2)
        load_x_granule(0, 8, KQ)
        load_x_granule(1, 8, KQ)
        load_w_piece(0, 3, 2)
        load_x_granule(0, 12, KQ)
        load_x_granule(1, 12, KQ)
        load_x_granule(1, 0, KQ)  # mc1's rotated k0-3 tail is consumed last
        # Panel 1 then chunks 2-3: panel-1 triggers wait on panel-0 staging
        # WARs, so everything from here is throttled behind the opening.
        for q in range(NKQ):
            load_w_piece(1, q, 1)
        for mc in range(2, MC):
            for k0 in range(0, KT, KQ):
                load_x_granule(mc, k0, KQ)

        evac_n = 0

        def evac(pm, t, sub, mc, last=False):
            # Copies on ACT (DVE only for the final drain); y stores ALL on
            # the sync ring so no y-trigger ring-drain can ever sit ahead of
            # a copy in the ACT queue and stall a PSUM bank handoff.
            nonlocal evac_n
            yo = yo_pool.tile([P, 512], mybir.dt.float32, tag="yo")
            if last and mc % 2 == 1:
                nc.vector.tensor_copy(yo[:], pm[:])
            else:
                nc.scalar.copy(yo[:], pm[:])
            evac_n += 1
            # Final drain splits across both HWDGE rings (the ACT ring is
            # otherwise DMA-free, so no drain can block the copies).
            ydma = nc.scalar.dma_start if (last and mc % 2 == 1) else nc.sync.dma_start
            ydma(
                out=yt[(t * SUBS + sub) * P : (t * SUBS + sub + 1) * P, bass.ts(mc, 512)],
                in_=yo[:],
            )

        def pm_tile(bank):
            return pm_pool.tile(
                [P, 512], mybir.dt.float32, tag=f"pm{bank}", name=f"pm{bank}"
            )

        def mm(pm, t, sub, mc, k, start=None, stop=None):
            q, kk = k // KQ, k % KQ
            nc.tensor.matmul(
                pm[:],
                wm_t[t][q][:, kk * n_panel + sub * P : kk * n_panel + (sub + 1) * P],
                xt[:, xsl(mc, k)],
                start=(k == 0) if start is None else start,
                stop=(k == KT - 1) if stop is None else stop,
            )

        # Panel 0, opening phase: k-major over x chunks 0,1 and all subs
        # (8 live groups; bank mc*4+sub). Spreading the x-chunk cost over
        # 2x the flops keeps phase demand (~260 GB/s) under the per-core
        # HBM rate, which a single-chunk phase (~380 GB/s) exceeds. The
        # mc1 groups accumulate k in rotated order (k4..15 then k0..3, a
        # PSUM group is k-order-agnostic), so the first k-window only
        # needs chunk 0's granule and the phase ramps with the stream.
        pmsA = {(sub, mc): pm_tile(mc * 4 + sub) for sub in range(SUBS) for mc in (0, 1)}
        for k in range(KT):
            for sub in range(SUBS):
                mm(pmsA[(sub, 0)], 0, sub, 0, k)
                if k >= KQ:
                    mm(pmsA[(sub, 1)], 0, sub, 1, k, start=(k == KQ), stop=False)
        for k in range(KQ):
            for sub in range(SUBS):
                mm(pmsA[(sub, 1)], 0, sub, 1, k, start=False, stop=(k == KQ - 1))
        for mc in (0, 1):
            for sub in range(SUBS):
                evac(pmsA[(sub, mc)], 0, sub, mc)
        # Chunks 2,3 on the now-resident panel-0 weights; phase mc uses the
        # banks of opening-phase chunk mc-2, in its evac order.
        for mc in range(2, MC):
            pms = {sub: pm_tile((mc - 2) * 4 + sub) for sub in range(SUBS)}
            for k in range(KT):
                for sub in range(SUBS):
                    mm(pms[sub], 0, sub, mc, k)
            for sub in range(SUBS):
                evac(pms[sub], 0, sub, mc)

        # Panels 1-3: sub-major sweeps; sub -> banks (sub%2)*4+mc.
        for t in range(1, NTP):
            if t + 1 <= NTP - 1:
                for q in range(NKQ):
                    load_w_piece(t + 1, q, 1)
            for sub in range(SUBS):
                last = t == NTP - 1 and sub == SUBS - 1
                pms = {mc: pm_tile((sub % 2) * 4 + mc) for mc in range(MC)}
                for k in range(KT):
                    for mc in range(MC):
                        mm(pms[mc], t, sub, mc, k)
                for mc in range(MC):
                    evac(pms[mc], t, sub, mc, last=last)

    nc.compile()
    return nc


def _prep_host(input_, weight, mask, n_panel=512):
    in_dim, out_dim = weight.shape[1], weight.shape[0]
    kt = in_dim // P
    ntp = out_dim // n_panel
    # Pre-tiled [t, p, kt*n]: wtp[t, p, kt*n_panel + n] = W^T[kt*P+p, t*n_panel+n]
    wtp = np.ascontiguousarray(
        weight.T.reshape(kt, P, ntp, n_panel).transpose(2, 1, 0, 3).reshape(
            ntp, P, kt * n_panel
        )
    ).astype(ml_dtypes.bfloat16)
    mkp = np.ascontiguousarray(
        mask.T.reshape(kt, P, ntp, n_panel).transpose(2, 1, 0, 3).reshape(
            ntp, P, kt * n_panel
        )
    ).astype(np.int8)
    rows = input_.shape[0] // N_CORES
    mc = rows // 512
    in_maps = []
    for c in range(N_CORES):
        # x chunk-major: xp[p, (mc*kt + k)*512 + m] = x^T[k*P+p, mc*512+m]
        xp = np.ascontiguousarray(
            input_[c * rows : (c + 1) * rows]
            .T.reshape(kt, P, mc, 512)
            .transpose(1, 2, 0, 3)
            .reshape(P, mc * kt * 512)
        ).astype(ml_dtypes.bfloat16)
        in_maps.append({"x": xp, "wt": wtp, "mk": mkp})
    return in_maps


_CACHE = {}


def _run(input_, weight, mask, trace=False, **build_kw):
    rows_total, in_dim = input_.shape
    out_dim = weight.shape[0]
    key = (rows_total, in_dim, out_dim, tuple(sorted(build_kw.items())))
    if key not in _CACHE:
        _CACHE[key] = build_nc(
            rows=rows_total // N_CORES, in_dim=in_dim, out_dim=out_dim, **build_kw
        )
    nc = _CACHE[key]
    in_maps = _prep_host(input_, weight, mask, build_kw.get("n_panel", 512))
    res = run_bass_kernel_spmd(nc, in_maps, core_ids=list(range(N_CORES)), trace=trace)
    out = np.concatenate(
        [np.ascontiguousarray(res.results[c]["yt"].T) for c in range(N_CORES)], axis=0
    )
    return out, res


def kernel(input_, weight, mask):
    input_ = np.asarray(input_, dtype=np.float32)
    weight = np.asarray(weight, dtype=np.float32)
    mask = np.asarray(mask)
    out, _ = _run(input_, weight, mask, trace=False)
    return out
